# revision 4
# baseline (speedup 1.0000x reference)
# kernel.py — self-contained Trainium2 Bass kernel for nn_AttnReadout
# Sharding: graph-level data parallel. Device d gets 512 contiguous graphs
# (131072 nodes). BN stats via per-device partial sums + AllReduce.
# sigmoid(y) computed as 0.5 + 0.5*tanh(y/2) so the whole inner loop stays
# on one ACT table set (tanh+exp coexist in exp_and_others).
#
# Wire format: feat ships as int8 with a per-node f32 scale
# (s = absmax(row)/127) — 129MB instead of 512MB over the axon tunnel,
# which is the end-to-end bottleneck (~65MB/s). Tiles are dequantized to
# f32 on device right after DMA; the rest of the pipeline is unchanged.
# The jit-sharded executable is built once and cached; per-run host work
# is limited to handing the prepacked arrays to PJRT.
import os
import sys

sys.path.insert(0, "/opt/trn_rl_repo")
os.environ["JAX_PLATFORMS"] = "axon"

import numpy as np

NUM_GRAPHS = 4096
NODES_PER_GRAPH = 256
N_TOTAL = NUM_GRAPHS * NODES_PER_GRAPH
IN_DIM = 128
HID_DIM = 128
OUT_DIM = 256
BN_EPS = 1e-5
N_CORES = 8

G_CORE = NUM_GRAPHS // N_CORES            # 512 graphs
N_CORE = G_CORE * NODES_PER_GRAPH         # 131072 nodes
CHUNK = 128
BLK_CHUNKS = 4                             # 512 nodes / block = 2 graphs
BLK_NODES = CHUNK * BLK_CHUNKS
GRAPHS_PER_BLK = BLK_NODES // NODES_PER_GRAPH

_CACHE = {}


def build_nc(n_cores, g_core):
    import concourse.bass as bass
    import concourse.bacc as bacc
    import concourse.tile as tile
    from concourse import mybir
    from concourse.masks import make_identity

    key = ("nc", n_cores, g_core)
    if key in _CACHE:
        return _CACHE[key]

    f32 = mybir.dt.float32
    i8 = mybir.dt.int8
    nc = bacc.Bacc("TRN2", target_bir_lowering=False, debug=False,
                   enable_asserts=False, num_devices=n_cores)
    n_core = g_core * NODES_PER_GRAPH
    feat = nc.dram_tensor("feat", [n_core, IN_DIM], i8, kind="ExternalInput")
    fscale = nc.dram_tensor("fscale", [n_core], f32, kind="ExternalInput")
    flast = nc.dram_tensor("flast", [g_core, IN_DIM], f32, kind="ExternalInput")
    W_u = nc.dram_tensor("W_u", [IN_DIM, HID_DIM], f32, kind="ExternalInput")
    W_v = nc.dram_tensor("W_v", [IN_DIM, HID_DIM], f32, kind="ExternalInput")
    b_v = nc.dram_tensor("b_v", [HID_DIM], f32, kind="ExternalInput")
    w_e = nc.dram_tensor("w_e", [HID_DIM, 1], f32, kind="ExternalInput")
    W_out = nc.dram_tensor("W_out", [IN_DIM, OUT_DIM], f32, kind="ExternalInput")
    gamma = nc.dram_tensor("gamma", [IN_DIM], f32, kind="ExternalInput")
    beta = nc.dram_tensor("beta", [IN_DIM], f32, kind="ExternalInput")
    rst = nc.dram_tensor("rst", [g_core, OUT_DIM], f32, kind="ExternalOutput")

    with tile.TileContext(nc) as tc:
        _emit(nc, tc, bass, tile, mybir, make_identity,
              feat, fscale, flast, W_u, W_v, b_v, w_e, W_out, gamma, beta,
              rst, n_cores, g_core)
    nc.compile()
    _CACHE[key] = nc
    return nc


def _emit(nc, tc, bass, tile, mybir, make_identity,
          feat, fscale, flast, W_u, W_v, b_v, w_e, W_out, gamma, beta, rst,
          n_cores, g_core):
    from contextlib import ExitStack

    f32 = mybir.dt.float32
    i8 = mybir.dt.int8
    AF = mybir.ActivationFunctionType
    ts = bass.ts
    n_core = g_core * NODES_PER_GRAPH
    n_total = n_core * n_cores
    n_blks = n_core // BLK_NODES

    ctx = ExitStack()
    with ctx:
        consts = ctx.enter_context(tc.tile_pool(name="consts", bufs=1))
        ident = consts.tile([128, 128], f32)
        make_identity(nc, ident[:])
        ones_col = consts.tile([128, 1], f32)
        nc.vector.memset(ones_col[:], 1.0)
        ones_row = consts.tile([1, 128], f32)
        nc.vector.memset(ones_row[:], 1.0)

        # ---------------- Phase A: BN stats ----------------
        # 16-chunk (2048-node) groups; int8 + per-node scale dequant to f32
        GRPC = 16
        feat_g = feat[:, :].rearrange("(ng c p) i -> ng p c i",
                                      p=CHUNK, c=GRPC)
        fs_g = fscale[:].rearrange("(ng c p) -> ng p c", p=CHUNK, c=GRPC)
        n_grps = n_core // (CHUNK * GRPC)
        with tc.tile_pool(name="pa_q", bufs=4) as pa_q, \
             tc.tile_pool(name="pa_s", bufs=4) as pa_s, \
             tc.tile_pool(name="pa_sb", bufs=3) as pa_sb, \
             tc.tile_pool(name="pa_sq", bufs=3) as pa_sq, \
             tc.tile_pool(name="pa_ps", bufs=1, space="PSUM") as pa_ps:
            ps_sum = pa_ps.tile([1, BLK_CHUNKS * IN_DIM], f32, tag="sum")
            ps_sq = pa_ps.tile([1, BLK_CHUNKS * IN_DIM], f32, tag="sq")
            for ng in range(n_grps):
                qt = pa_q.tile([128, GRPC, IN_DIM], i8)
                nc.sync.dma_start(qt[:], feat_g[ng])
                st = pa_s.tile([128, GRPC], f32)
                nc.sync.dma_start(st[:], fs_g[ng])
                ft = pa_sb.tile([128, GRPC, IN_DIM], f32)
                for c in range(GRPC):
                    nc.vector.tensor_scalar_mul(ft[:, c, :], qt[:, c, :],
                                                st[:, c:c + 1])
                sq = pa_sq.tile([128, GRPC, IN_DIM], f32)
                nc.scalar.square(sq[:], ft[:])
                for j in range(GRPC // BLK_CHUNKS):
                    first = (ng == 0 and j == 0)
                    last = (ng == n_grps - 1 and j == GRPC // BLK_CHUNKS - 1)
                    sl = slice(j * BLK_CHUNKS, (j + 1) * BLK_CHUNKS)
                    nc.tensor.matmul(ps_sum[:], ones_col[:], ft[:, sl, :],
                                     start=first, stop=last,
                                     skip_group_check=True)
                    nc.tensor.matmul(ps_sq[:], ones_col[:], sq[:, sl, :],
                                     start=first, stop=last,
                                     skip_group_check=True)
            stats_sb = consts.tile([1, 1024], f32, tag="stats")
            nc.vector.tensor_copy(stats_sb[:, 0:512], ps_sum[:])
            nc.vector.tensor_copy(stats_sb[:, 512:1024], ps_sq[:])

        # ---------------- AllReduce of stats ----------------
        gstats = consts.tile([1, 1024], f32, tag="gstats")
        if n_cores > 1:
            with tc.tile_pool(name="dram", bufs=1, space="DRAM") as dram:
                cin = dram.tile([1, 1024], f32, tag="cin")
                cout = dram.tile([1, 1024], f32, tag="cout")
                nc.gpsimd.dma_start(cin[:], stats_sb[:])
                nc.gpsimd.collective_compute(
                    "AllReduce", mybir.AluOpType.add,
                    replica_groups=[list(range(n_cores))],
                    ins=[cin.opt()], outs=[cout.opt()])
                nc.gpsimd.dma_start(gstats[:], cout[:])
        else:
            nc.vector.tensor_copy(gstats[:], stats_sb[:])

        # fold 4 sub-chunk partials -> [1,128]; a = gamma*rsqrt(var+eps),
        # b = beta - mean*a
        srow = consts.tile([1, 128], f32, tag="srow")
        qrow = consts.tile([1, 128], f32, tag="qrow")
        t0 = consts.tile([1, 128], f32, tag="t0")
        t1 = consts.tile([1, 128], f32, tag="t1")
        nc.vector.tensor_add(t0[:], gstats[:, 0:128], gstats[:, 128:256])
        nc.vector.tensor_add(t1[:], gstats[:, 256:384], gstats[:, 384:512])
        nc.vector.tensor_add(srow[:], t0[:], t1[:])
        nc.vector.tensor_add(t0[:], gstats[:, 512:640], gstats[:, 640:768])
        nc.vector.tensor_add(t1[:], gstats[:, 768:896], gstats[:, 896:1024])
        nc.vector.tensor_add(qrow[:], t0[:], t1[:])

        mean_r = consts.tile([1, 128], f32, tag="mean")
        ex2_r = consts.tile([1, 128], f32, tag="ex2")
        nc.scalar.mul(mean_r[:], srow[:], 1.0 / n_total)
        nc.scalar.mul(ex2_r[:], qrow[:], 1.0 / n_total)
        var_r = consts.tile([1, 128], f32, tag="var")
        nc.vector.tensor_mul(t0[:], mean_r[:], mean_r[:])
        nc.vector.tensor_scalar_mul(t0[:], t0[:], -1.0)
        nc.vector.tensor_add(var_r[:], t0[:], ex2_r[:])
        eps_t = consts.tile([1, 1], f32, tag="eps")
        nc.vector.memset(eps_t[:], BN_EPS)
        sd_r = consts.tile([1, 128], f32, tag="sd")
        nc.scalar.activation(sd_r[:], var_r[:], AF.Sqrt, bias=eps_t[:], scale=1.0)
        rs_r = consts.tile([1, 128], f32, tag="rs")
        nc.vector.reciprocal(rs_r[:], sd_r[:])

        grow = consts.tile([1, 128], f32, tag="grow")
        brow = consts.tile([1, 128], f32, tag="brow")
        nc.sync.dma_start(grow[:], gamma[:].rearrange("(o p) -> o p", o=1))
        nc.sync.dma_start(brow[:], beta[:].rearrange("(o p) -> o p", o=1))
        a_r = consts.tile([1, 128], f32, tag="a_r")
        b_r = consts.tile([1, 128], f32, tag="b_r")
        nc.vector.tensor_mul(a_r[:], rs_r[:], grow[:])
        nc.vector.tensor_mul(t0[:], mean_r[:], a_r[:])
        nc.vector.tensor_scalar_mul(t0[:], t0[:], -1.0)
        nc.vector.tensor_add(b_r[:], t0[:], brow[:])

        # folded weights + per-graph bias matrix vT (scaled by 0.5 for tanh)
        with tc.tile_pool(name="prep_ps", bufs=1, space="PSUM") as prep_ps, \
             tc.tile_pool(name="flt", bufs=2) as flt_pool:
            aT = consts.tile([128, 1], f32, tag="aT")
            bT = consts.tile([128, 1], f32, tag="bT")
            pT = prep_ps.tile([128, 1], f32, tag="pT")
            nc.tensor.transpose(pT[:], a_r[:], ident[0:1, 0:1])
            nc.vector.tensor_copy(aT[:], pT[:])
            pT2 = prep_ps.tile([128, 1], f32, tag="pT2")
            nc.tensor.transpose(pT2[:], b_r[:], ident[0:1, 0:1])
            nc.vector.tensor_copy(bT[:], pT2[:])

            Wu_sb = consts.tile([128, HID_DIM], f32, tag="Wu")
            Wv_sb = consts.tile([128, HID_DIM], f32, tag="Wv")
            Wout_sb = consts.tile([128, OUT_DIM], f32, tag="Wout")
            we_sb = consts.tile([128, 1], f32, tag="we")
            bv_col = consts.tile([128, 1], f32, tag="bv")
            nc.sync.dma_start(Wu_sb[:], W_u[:, :])
            nc.sync.dma_start(Wv_sb[:], W_v[:, :])
            nc.sync.dma_start(Wout_sb[:], W_out[:, :])
            nc.sync.dma_start(we_sb[:], w_e[:, :])
            nc.sync.dma_start(bv_col[:], b_v[:].rearrange("(p o) -> p o", o=1))

            Wu_s = consts.tile([128, HID_DIM], f32, tag="Wu_s")
            Wv_s = consts.tile([128, HID_DIM], f32, tag="Wv_s")
            nc.vector.tensor_scalar_mul(Wu_s[:], Wu_sb[:], aT[:])
            nc.vector.tensor_scalar_mul(Wv_s[:], Wv_sb[:], aT[:])

            # we_h = 0.5*w_e ; c0b = 0.5*sum(w_e) broadcast column
            we_h = consts.tile([128, 1], f32, tag="we_h")
            nc.scalar.mul(we_h[:], we_sb[:], 0.5)
            c0_ps = prep_ps.tile([1, 1], f32, tag="c0")
            nc.tensor.matmul(c0_ps[:], we_sb[:], ones_col[:], start=True, stop=True)
            c0_sb = consts.tile([1, 1], f32, tag="c0_sb")
            nc.scalar.mul(c0_sb[:], c0_ps[:], 0.5)
            c0b_ps = prep_ps.tile([128, 1], f32, tag="c0b")
            nc.tensor.matmul(c0b_ps[:], ones_row[:], c0_sb[:], start=True, stop=True)
            c0b = consts.tile([128, 1], f32, tag="c0b_sb")
            nc.vector.tensor_copy(c0b[:], c0b_ps[:])

            cu_ps = prep_ps.tile([128, 1], f32, tag="cu")
            nc.tensor.matmul(cu_ps[:], Wu_sb[:], bT[:], start=True, stop=True)
            cu_sb = consts.tile([128, 1], f32, tag="cu_sb")
            nc.vector.tensor_copy(cu_sb[:], cu_ps[:])
            cv_ps = prep_ps.tile([128, 1], f32, tag="cv")
            nc.tensor.matmul(cv_ps[:], Wv_sb[:], bT[:], start=True, stop=True)
            tb_sb = consts.tile([128, 1], f32, tag="tb")
            nc.scalar.add(tb_sb[:], cv_ps[:], bv_col[:])
            nc.vector.tensor_add(tb_sb[:], tb_sb[:], cu_sb[:])

            vT_sb = consts.tile([128, g_core], f32, tag="vT")
            fl_r = flast[:, :].rearrange("(c p) i -> c p i", p=128)
            for c in range(g_core // 128):
                flc = flt_pool.tile([128, IN_DIM], f32)
                nc.sync.dma_start(flc[:], fl_r[c])
                flT_ps = prep_ps.tile([128, 128], f32, tag="flT")
                nc.tensor.transpose(flT_ps[:], flc[:], ident[:])
                flT_sb = flt_pool.tile([128, 128], f32, tag="flT_sb")
                nc.vector.tensor_copy(flT_sb[:], flT_ps[:])
                vps = prep_ps.tile([128, 128], f32, tag="vps")
                nc.tensor.matmul(vps[:], Wv_s[:], flT_sb[:], start=True, stop=True)
                nc.scalar.add(vT_sb[:, ts(c, 128)], vps[:], tb_sb[:])
            # scale by 0.5 for the tanh form of sigmoid
            nc.vector.tensor_scalar_mul(vT_sb[:], vT_sb[:], 0.5)

        # ---------------- Phase B: main pass ----------------
        # Pool with UNNORMALIZED exp weights into one device-wide PSUM bank;
        # 1/z and the +b fold are applied after W_out where layout is row-major.
        feat_r = feat[:, :].rearrange("(nb c p) i -> nb p c i",
                                      p=CHUNK, c=BLK_CHUNKS)
        fs_r = fscale[:].rearrange("(nb c p) -> nb p c", p=CHUNK, c=BLK_CHUNKS)
        with tc.tile_pool(name="ps_pz", bufs=1, space="PSUM") as ps_pz, \
             tc.tile_pool(name="ps_z", bufs=1, space="PSUM") as ps_z:
          PZ = ps_pz.tile([128, g_core], f32)
          Z = ps_z.tile([1, g_core], f32)
          with tc.tile_pool(name="pb_q", bufs=4) as pb_q, \
               tc.tile_pool(name="pb_s", bufs=4) as pb_s, \
               tc.tile_pool(name="pb_feat", bufs=4) as pb_feat, \
               tc.tile_pool(name="pb_sb", bufs=3) as pb_sb, \
               tc.tile_pool(name="pb_w", bufs=3) as pb_w, \
               tc.tile_pool(name="ps_ft", bufs=2, space="PSUM") as ps_ft, \
               tc.tile_pool(name="ps_u", bufs=2, space="PSUM") as ps_u, \
               tc.tile_pool(name="ps_e", bufs=2, space="PSUM") as ps_e:
            for nb in range(n_blks):
                qt = pb_q.tile([128, BLK_CHUNKS, IN_DIM], i8)
                nc.sync.dma_start(qt[:], feat_r[nb])
                st = pb_s.tile([128, BLK_CHUNKS], f32)
                nc.sync.dma_start(st[:], fs_r[nb])
                ft = pb_feat.tile([128, BLK_CHUNKS, IN_DIM], f32)
                for c in range(BLK_CHUNKS):
                    nc.vector.tensor_scalar_mul(ft[:, c, :], qt[:, c, :],
                                                st[:, c:c + 1])
                fT_ps = ps_ft.tile([128, BLK_NODES], f32)
                for c in range(BLK_CHUNKS):
                    nc.tensor.transpose(fT_ps[:, ts(c, 128)], ft[:, c, :],
                                        ident[:])
                fT_sb = pb_sb.tile([128, BLK_NODES], f32, tag="fT")
                nc.vector.tensor_copy(fT_sb[:], fT_ps[:])
                uT_ps = ps_u.tile([128, BLK_NODES], f32)
                nc.tensor.matmul(uT_ps[:], Wu_s[:], fT_sb[:],
                                 start=True, stop=True)
                sigT = pb_sb.tile([128, BLK_NODES], f32, tag="sigT")
                for gb in range(GRAPHS_PER_BLK):
                    g = nb * GRAPHS_PER_BLK + gb
                    nc.scalar.activation(
                        sigT[:, ts(gb, NODES_PER_GRAPH)],
                        uT_ps[:, ts(gb, NODES_PER_GRAPH)],
                        AF.Tanh, bias=vT_sb[:, g:g + 1], scale=0.5)
                eT_ps = ps_e.tile([128, BLK_CHUNKS], f32)
                for c in range(BLK_CHUNKS):
                    nc.tensor.matmul(eT_ps[:, c:c + 1], sigT[:, ts(c, 128)],
                                     we_h[:], start=True, stop=True)
                wT = pb_w.tile([128, BLK_CHUNKS], f32, tag="wT")
                nc.scalar.activation(wT[:], eT_ps[:], AF.Exp,
                                     bias=c0b[:], scale=1.0)
                for gb in range(GRAPHS_PER_BLK):
                    g = nb * GRAPHS_PER_BLK + gb
                    for r in range(2):
                        cc = gb * 2 + r
                        nc.tensor.matmul(Z[0:1, g:g + 1], ones_col[:],
                                         wT[:, cc:cc + 1],
                                         start=(r == 0), stop=(r == 1),
                                         skip_group_check=True)
                        nc.tensor.matmul(PZ[:, g:g + 1], ft[:, cc, :],
                                         wT[:, cc:cc + 1],
                                         start=(r == 0), stop=(r == 1),
                                         skip_group_check=True)

          # ---------------- Tail: W_out + 1/z + output ----------------
          with tc.tile_pool(name="tail_sb", bufs=2) as tail_sb, \
               tc.tile_pool(name="tail_ps", bufs=1, space="PSUM") as tail_ps:
              poolRaw = consts.tile([128, g_core], f32, tag="poolRaw")
              nc.vector.tensor_copy(poolRaw[:], PZ[:])
              zrow = consts.tile([1, g_core], f32, tag="zrow")
              nc.vector.tensor_copy(zrow[:], Z[:])
              rz_row = consts.tile([1, g_core], f32, tag="rz_row")
              nc.vector.reciprocal(rz_row[:], zrow[:])

              # W_out folded with a;  c_out = b @ W_out broadcast to rows
              Wout_a = consts.tile([128, OUT_DIM], f32, tag="Wout_a")
              nc.vector.tensor_scalar_mul(Wout_a[:], Wout_sb[:], aT[:])
              co_ps = tail_ps.tile([128, 2], f32, tag="co")
              for h in range(2):
                  nc.tensor.matmul(co_ps[:, h:h + 1], Wout_sb[:, ts(h, 128)],
                                   bT[:], start=True, stop=True)
              co_sb = consts.tile([128, 2], f32, tag="co_sb")
              nc.vector.tensor_copy(co_sb[:], co_ps[:])
              cor_ps = tail_ps.tile([1, 2, 128], f32, tag="cor")
              for h in range(2):
                  nc.tensor.transpose(cor_ps[:, h, :], co_sb[:, h:h + 1],
                                      ident[:])
              co_row = consts.tile([1, 2, 128], f32, tag="co_row")
              nc.vector.tensor_copy(co_row[:], cor_ps[:])
              cob_ps = tail_ps.tile([128, 2, 128], f32, tag="cob")
              nc.tensor.matmul(cob_ps[:], ones_row[:],
                               co_row[:].rearrange("o h d -> o (h d)"),
                               start=True, stop=True)
              co_bc = consts.tile([128, 2, 128], f32, tag="co_bc")
              nc.vector.tensor_copy(co_bc[:], cob_ps[:])

              rstT_sb = []
              for h in range(2):
                  rp = tail_ps.tile([128, g_core], f32, tag="rstT")
                  nc.tensor.matmul(rp[:], Wout_a[:, ts(h, 128)], poolRaw[:],
                                   start=True, stop=True)
                  rs_sb = tail_sb.tile([128, g_core], f32, tag="rstT_sb")
                  nc.vector.tensor_copy(rs_sb[:], rp[:])
                  rstT_sb.append(rs_sb)
              rst_r = rst[:, :].rearrange("(gc p) o -> gc p o", p=128)
              for gc in range(g_core // 128):
                  rzT_ps = tail_ps.tile([128, 1], f32, tag="rzT")
                  nc.tensor.transpose(rzT_ps[:], rz_row[:, ts(gc, 128)],
                                      ident[0:1, 0:1])
                  rzT = tail_sb.tile([128, 1], f32, tag="rzT_sb")
                  nc.vector.tensor_copy(rzT[:], rzT_ps[:])
                  rt_ps = tail_ps.tile([128, 2, 128], f32, tag="rt")
                  for h in range(2):
                      nc.tensor.transpose(rt_ps[:, h, :],
                                          rstT_sb[h][:, ts(gc, 128)],
                                          ident[:])
                  rt_sb = tail_sb.tile([128, 2, 128], f32, tag="rt_sb")
                  nc.vector.tensor_scalar_mul(rt_sb[:], rt_ps[:], rzT[:])
                  nc.vector.tensor_add(rt_sb[:], rt_sb[:], co_bc[:])
                  nc.sync.dma_start(rst_r[gc],
                                    rt_sb[:].rearrange("p h o -> p (h o)"))


# ---------------- cached PJRT runner ----------------

def _build_runner(n_cores, g_core):
    """Build the jit-sharded executable once; mirrors
    bass2jax.run_bass_via_pjrt but reusable across calls (no per-call
    retrace/recompile) and fed with full global arrays (no host concat)."""
    key = ("runner", n_cores, g_core)
    if key in _CACHE:
        return _CACHE[key]

    import jax
    from jax.sharding import Mesh, PartitionSpec
    from jax.experimental.shard_map import shard_map
    from concourse import bass2jax, mybir

    nc = build_nc(n_cores, g_core)
    bass2jax.install_neuronx_cc_hook()
    partition_name = (nc.partition_id_tensor.name
                      if nc.partition_id_tensor else None)

    in_names, out_names, out_avals, out_shapes = [], [], [], []
    for alloc in nc.m.functions[0].allocations:
        if not isinstance(alloc, mybir.MemoryLocationSet):
            continue
        name = alloc.memorylocations[0].name
        if alloc.kind == "ExternalInput":
            if name != partition_name:
                in_names.append(name)
        elif alloc.kind == "ExternalOutput":
            shape = tuple(alloc.tensor_shape)
            dtype = mybir.dt.np(alloc.dtype)
            out_names.append(name)
            out_avals.append(jax.core.ShapedArray(shape, dtype))
            out_shapes.append((shape, dtype))
    n_params = len(in_names)
    n_outs = len(out_names)
    in_names_all = list(in_names) + list(out_names)
    if partition_name is not None:
        in_names_all.append(partition_name)

    def _body(*args):
        operands = list(args)
        if partition_name is not None:
            operands.append(bass2jax.partition_id_tensor())
        outs = bass2jax._bass_exec_p.bind(
            *operands,
            out_avals=tuple(out_avals),
            in_names=tuple(in_names_all),
            out_names=tuple(out_names),
            lowering_input_output_aliases=(),
            sim_require_finite=True,
            sim_require_nnan=True,
            nc=nc,
        )
        return tuple(outs)

    devices = jax.devices()[:n_cores]
    assert len(devices) == n_cores
    mesh = Mesh(np.asarray(devices), ("core",))
    in_specs = (PartitionSpec("core"),) * (n_params + n_outs)
    out_specs = (PartitionSpec("core"),) * n_outs
    donate = tuple(range(n_params, n_params + n_outs))
    sharded = jax.jit(
        shard_map(_body, mesh=mesh, in_specs=in_specs, out_specs=out_specs,
                  check_rep=False),
        donate_argnums=donate, keep_unused=True)

    runner = {"sharded": sharded, "in_names": in_names,
              "out_names": out_names, "out_shapes": out_shapes,
              "n_cores": n_cores}
    _CACHE[key] = runner
    return runner


def quantize_feat(feat):
    """Per-node symmetric int8: s = absmax(row)/127; q = rint(feat/s).
    Chunk-threaded — numpy releases the GIL inside the ufunc loops."""
    from concurrent.futures import ThreadPoolExecutor
    n = feat.shape[0]
    q = np.empty(feat.shape, np.int8)
    s = np.empty((n,), np.float32)
    n_chunks = 16
    step = (n + n_chunks - 1) // n_chunks

    def work(i):
        lo, hi = i * step, min((i + 1) * step, n)
        blk = feat[lo:hi]
        sm = np.max(np.abs(blk), axis=1)
        np.maximum(sm, 1e-30, out=sm)
        sm /= 127.0
        s[lo:hi] = sm
        tmp = blk * (1.0 / sm)[:, None]
        np.rint(tmp, out=tmp)
        q[lo:hi] = tmp.astype(np.int8)

    with ThreadPoolExecutor(max_workers=8) as ex:
        list(ex.map(work, range(n_chunks)))
    return q, s


def make_bundle(feat, flast_full, W_u, W_v, b_v, w_e, W_out, gamma, beta,
                n_cores=N_CORES):
    """Pack global (concatenated-across-cores) arrays in the runner's
    input order. Replicated small tensors are tiled n_cores times."""
    q, s = quantize_feat(np.ascontiguousarray(feat, np.float32))
    rep = lambda a: np.tile(np.ascontiguousarray(a, np.float32),
                            (n_cores,) + (1,) * (a.ndim - 1))
    return {
        "feat": q,
        "fscale": s,
        "flast": np.ascontiguousarray(flast_full, np.float32),
        "W_u": rep(W_u), "W_v": rep(W_v), "b_v": rep(b_v),
        "w_e": rep(w_e), "W_out": rep(W_out),
        "gamma": rep(gamma), "beta": rep(beta),
    }


def run_full(bundle, n_cores=N_CORES, g_core=G_CORE):
    """One warm run: hand global arrays to the cached executable."""
    runner = _build_runner(n_cores, g_core)
    args = [bundle[name] for name in runner["in_names"]]
    zeros = [np.zeros((n_cores * sh[0], *sh[1:]), dt)
             for sh, dt in runner["out_shapes"]]
    outs = runner["sharded"](*args, *zeros)
    return np.asarray(outs[0])  # global rst == full [NUM_GRAPHS, OUT_DIM]


def run_cores(in_maps, n_cores, g_core, trace=False):
    """Back-compat per-core-map entry (concatenates then run_full)."""
    runner = _build_runner(n_cores, g_core)
    bundle = {name: np.concatenate([np.asarray(m[name]) for m in in_maps],
                                   axis=0)
              for name in runner["in_names"]}
    out = run_full(bundle, n_cores, g_core)

    class R:
        pass
    r = R()
    r.results = [{"rst": out[d * g_core:(d + 1) * g_core]}
                 for d in range(n_cores)]
    return r


def _numpy_fallback(feat, gamma, beta, W_u, W_v, b_v, w_e, W_out,
                    segment_ids, last_nodes):
    mean = feat.mean(0)
    var = ((feat - mean) ** 2).mean(0)
    x = (feat - mean) / np.sqrt(var + BN_EPS) * gamma + beta
    fu = x @ W_u
    fv = x[last_nodes] @ W_v + b_v
    e = (1.0 / (1.0 + np.exp(-(fu + fv[segment_ids]))) @ w_e)[:, 0]
    G = int(segment_ids.max()) + 1
    m = np.full(G, -np.inf, np.float32)
    np.maximum.at(m, segment_ids, e)
    ex = np.exp(e - m[segment_ids])
    z = np.zeros(G, np.float32)
    np.add.at(z, segment_ids, ex)
    alpha = ex / z[segment_ids]
    rstv = np.zeros((G, feat.shape[1]), np.float32)
    np.add.at(rstv, segment_ids, x * alpha[:, None])
    return (rstv @ W_out).astype(np.float32)


def kernel(**inputs):
    feat = np.ascontiguousarray(inputs["feat"], dtype=np.float32)
    seg = np.asarray(inputs["segment_ids"])
    last = np.asarray(inputs["last_nodes"])
    expected_seg = np.repeat(np.arange(NUM_GRAPHS, dtype=np.int64),
                             NODES_PER_GRAPH)
    if feat.shape != (N_TOTAL, IN_DIM) or \
            not np.array_equal(seg.astype(np.int64), expected_seg):
        return _numpy_fallback(
            np.asarray(inputs["feat"], np.float32),
            np.asarray(inputs["gamma"], np.float32),
            np.asarray(inputs["beta"], np.float32),
            np.asarray(inputs["W_u"], np.float32),
            np.asarray(inputs["W_v"], np.float32),
            np.asarray(inputs["b_v"], np.float32),
            np.asarray(inputs["w_e"], np.float32),
            np.asarray(inputs["W_out"], np.float32),
            seg.astype(np.int64), last.astype(np.int64))

    flast_full = np.ascontiguousarray(feat[last.astype(np.int64)])
    bundle = make_bundle(feat, flast_full,
                         inputs["W_u"], inputs["W_v"], inputs["b_v"],
                         inputs["w_e"], inputs["W_out"],
                         inputs["gamma"], inputs["beta"])
    out = run_full(bundle)
    return out.astype(np.float32)
